# revision 52
# baseline (speedup 1.0000x reference)
"""Trainium2 Bass kernel: ragged mean-pool over [1, len_i] + Linear->tanh->Linear head.

Strategy (pure data parallel over batch, 8 NeuronCores):
  * Host: balance the 256 samples across 8 cores (32 each) by row count (LPT),
    gather the rows hidden_states[b, 1:len_b+1, :] into a dense per-core pack
    of 128-row "slices" (row j -> slice j//128, partition j%128), and encode
    rows in fp8 e3m4 with error-feedback quantization chained along the
    partition axis inside each slice -- the carries cancel in the device-side
    per-sample sums (~0.6% pooled error vs ~1.1% plain RNE).
  * Device: stream the packed rows partition-major (per-partition-contiguous
    DMA descriptors reach ~360+ GB/s) in chunks sized small->big->small so the
    pooling matmuls start early and chase the last bytes closely. Pooling =
    PE matmuls with the 0/1 membership matrix (fp8) as stationary operand,
    split into THREE concurrent 256-column streams on disjoint PE column
    strips (col_grps 0/1/2 -> partitions 32s..32s+32 of one PSUM bank).
    Mean scale (1/len) is folded into the PSUM->SBUF copies (DVE
    tensor_scalar / ScalarE activation-scale, split across both engines).
    Head: PE transposes -> dense (fp8 e3m4 weights x64, dequant via the
    tanh's scale) -> tanh -> classifier (fp16) fully on-chip; tanh table
    preloaded at kernel start; logits stored fp16 and upcast on host.
  * Host: scatter per-core logits [32, 96] back to the full [256, 96].

Compiled program depends only on (n_slices, chunk split) -- raggedness lives
in the data (packing + membership), so recompiles are rare.
"""

import os
from contextlib import ExitStack

import numpy as np
import ml_dtypes

import concourse.bass as bass
import concourse.mybir as mybir
from concourse import bacc, bass_utils

B, S, H, T_OUT = 256, 512, 768, 96
N_CORES = 8
LOCAL_B = B // N_CORES        # 32 samples per core
F32 = mybir.dt.float32
F16 = mybir.dt.float16
F8 = mybir.dt.float8e3       # e3m4: 1-3-4, max 15.5, ~1.1% RMS quant err
NP_F8 = ml_dtypes.float8_e3m4

# COLT=2: column-tile the a-half pooling matmuls across two PE strips.
COLT = int(os.environ.get("KERNEL_COLT", "2"))
# DWT8=1: dense weights quantized to fp8 e3m4 (x64 pre-scale, 1/64 folded
# into the tanh's scale) and fed to a mixed fp8xfp16 dense matmul.
DWT8 = int(os.environ.get("KERNEL_DWT8", "1"))
MODE = f"f8e3-colt{COLT}-dwt8{DWT8}"  # informational (test.py prints it)

_cache: dict = {}
last_results = None  # BassKernelResults of the most recent run (for test.py)


def _chunk_split(n_slices: int) -> tuple:
    """Split n_slices into DMA chunks. The PE consumes a chunk only once the
    whole chunk has landed, so chunks shrink toward the end of the stream
    ([8, 4, 2, 2] tail) -- the pooling matmuls chase the last bytes closely
    instead of serializing a big chunk's matmul burst after the DMA."""
    head = [4, 8]
    tail = [8, 4, 2]
    if n_slices < sum(head) + sum(tail) + 8:
        out = []
        left = n_slices
        for c in [4] * (n_slices // 4) + [n_slices % 4]:
            if c:
                out.append(c)
        return tuple(out)
    rest = n_slices - sum(head) - sum(tail)
    n_big = max(1, round(rest / 13))
    base, extra = divmod(rest, n_big)
    return tuple(
        head + [base + (1 if i < extra else 0) for i in range(n_big)] + tail
    )


def _build_program(n_slices: int, colt: int, dwt8: int = 0) -> bass.Bass:
    chunks = _chunk_split(n_slices)
    n_chunks = len(chunks)
    chunk_start = np.cumsum([0] + list(chunks))[:-1]
    start_to_chunk = {int(s): i for i, s in enumerate(chunk_start)}

    # No collectives -> no partition id; skipping it drops 5 per-engine
    # TENSOR_LOADs (~2us) from the launch preamble.
    nc = bacc.Bacc(enable_partition_id=False, monotonic_sem_count=0)

    hsb_d = nc.declare_dram_parameter("hsb", [128, n_slices * H], F8, isOutput=False)
    member_d = nc.declare_dram_parameter(
        "member", [128, n_slices * LOCAL_B], F8, isOutput=False
    )
    # dwT jg-major: cols [jg*H + c*128 + j] = dense_w[jg*128+j, c*128+h]^T --
    # contiguous halves so the dense layer can start after half the transfer.
    WDT = F8 if dwt8 else F16
    dwT_d = nc.declare_dram_parameter("dwT", [128, 6 * H], WDT, isOutput=False)
    cwT_d = nc.declare_dram_parameter("cwT", [128, 6 * T_OUT], F16, isOutput=False)
    # identd = diag(1/len) fp16: the transpose's "identity" operand, folding
    # the per-sample mean scale into the PE transposes for free.
    identd_d = nc.declare_dram_parameter("identd", [96, LOCAL_B], F16, isOutput=False)
    smalls_d = nc.declare_dram_parameter("smalls", [128, 8], F32, isOutput=False)
    clsb_d = nc.declare_dram_parameter("clsb", [LOCAL_B, T_OUT], F32, isOutput=False)
    # fp16 store: logits are <1 in magnitude, fp16 rounding ~5e-5 relative;
    # halves the store's data phase. Host upcasts to f32.
    out_d = nc.declare_dram_parameter("out", [LOCAL_B, T_OUT], F16, isOutput=True)

    with ExitStack() as ctx:
        hs_sb = ctx.enter_context(nc.sbuf_tensor([128, n_slices * H], F8))
        member_t = ctx.enter_context(nc.sbuf_tensor([128, n_slices * LOCAL_B], F8))
        dwT_t = ctx.enter_context(nc.sbuf_tensor([128, 6 * H], WDT))
        cwT_t = ctx.enter_context(nc.sbuf_tensor([128, 6 * T_OUT], F16))
        identd_t = ctx.enter_context(nc.sbuf_tensor([96, LOCAL_B], F16))
        smalls_t = ctx.enter_context(nc.sbuf_tensor([128, 8], F32))
        clsb_t = ctx.enter_context(nc.sbuf_tensor([LOCAL_B, T_OUT], F32))
        # pooled_sb: a-half (cols 0:512) at partitions 0-31, b-half (cols
        # 512:768) at partitions 64-95 -- each engine copy keeps in/out on the
        # same partitions (no cross-lane moves), transposes pick the right
        # slice + identd block.
        pooled_sb = ctx.enter_context(nc.sbuf_tensor([96, H], F16))
        pooledT_sb = ctx.enter_context(nc.sbuf_tensor([128, 6 * LOCAL_B], F16))
        hT_sb = ctx.enter_context(nc.sbuf_tensor([128, 6 * LOCAL_B], F16))
        logits_sb = ctx.enter_context(nc.sbuf_tensor([LOCAL_B, T_OUT], F16))
        warm_sb = ctx.enter_context(nc.sbuf_tensor([128, 512], F8))
        scratch_sb = ctx.enter_context(nc.sbuf_tensor([128, 8], F32))

        # PSUM budget (8 banks): pooled, tp0-2, hps0-2, lps.
        # The pooling runs as THREE concurrent 256-column matmul streams on
        # disjoint PE column strips: strip s handles hidden cols
        # [256s, 256s+256) and accumulates into partitions [32s, 32s+32) of
        # one PSUM bank (107ns/slice issue pitch instead of 320).
        pooled = ctx.enter_context(nc.psum_tensor([96, 512], F32))
        tp = [
            ctx.enter_context(nc.psum_tensor(f"tp{i}", [128, 512], F16))
            for i in range(3)
        ]
        hps = [
            ctx.enter_context(nc.psum_tensor(f"hps{i}", [128, 512], F32))
            for i in range(3)
        ]
        lps = ctx.enter_context(nc.psum_tensor([LOCAL_B, 512], F32))

        db6_ap = smalls_t[:, 0:6]

        s_member = nc.alloc_semaphore("s_member")
        s_member1 = nc.alloc_semaphore("s_member1")
        s_chunk = [nc.alloc_semaphore(f"s_chunk{i}") for i in range(n_chunks)]
        s_smalls = nc.alloc_semaphore("s_smalls")
        s_dwTa = nc.alloc_semaphore("s_dwTa")
        s_dwTb = nc.alloc_semaphore("s_dwTb")
        s_cwT = nc.alloc_semaphore("s_cwT")
        s_warm = nc.alloc_semaphore("s_warm")
        s_pool = [nc.alloc_semaphore(f"s_pool{s}") for s in range(3)]
        s_sc = [nc.alloc_semaphore(f"s_sc{s}") for s in range(3)]
        s_tr = nc.alloc_semaphore("s_tr")
        s_ptcopy = nc.alloc_semaphore("s_ptcopy")
        s_head = nc.alloc_semaphore("s_head")
        s_tanh = nc.alloc_semaphore("s_tanh")
        s_cls = nc.alloc_semaphore("s_cls")
        s_log = nc.alloc_semaphore("s_log")
        s_out = nc.alloc_semaphore("s_out")

        with nc.Block() as block:

            @block.gpsimd
            def _(gpsimd):
                nc.gpsimd.memset(warm_sb[:], 0.0).then_inc(s_warm, 1)

            @block.sync
            def _(sync):
                # FIFO ring in consumption order. Head weights ride last: the
                # adds/transposes overlap their transfer, and only the dense
                # layer waits on them.
                # member front-slice first: the first pooling matmuls need
                # only the first chunk's member columns.
                m0 = min(chunks[0], n_slices) * LOCAL_B
                sync.dma_start(out=member_t[:, :m0], in_=member_d[:, :m0]).then_inc(
                    s_member, 16
                )
                for ci, (cs, cn) in enumerate(zip(chunk_start, chunks)):
                    sync.dma_start(
                        out=hs_sb[:, cs * H : (cs + cn) * H],
                        in_=hsb_d[:, cs * H : (cs + cn) * H],
                    ).then_inc(s_chunk[ci], 16)
                    if ci == 0:
                        sync.dma_start(
                            out=member_t[:, m0:], in_=member_d[:, m0:]
                        ).then_inc(s_member1, 16)
                        # tiny consts ride early (needed at pooling end)
                        sync.dma_start(out=smalls_t[:], in_=smalls_d[:]).then_inc(
                            s_smalls, 16
                        )
                        sync.dma_start(out=identd_t[:], in_=identd_d[:]).then_inc(
                            s_smalls, 16
                        )
                        sync.dma_start(out=clsb_t[:], in_=clsb_d[:]).then_inc(
                            s_smalls, 16
                        )
                sync.dma_start(
                    out=dwT_t[:, : 3 * H], in_=dwT_d[:, : 3 * H]
                ).then_inc(s_dwTa, 16)
                sync.dma_start(
                    out=dwT_t[:, 3 * H :], in_=dwT_d[:, 3 * H :]
                ).then_inc(s_dwTb, 16)
                sync.dma_start(out=cwT_t[:], in_=cwT_d[:]).then_inc(s_cwT, 16)
                sync.wait_ge(s_log, 1)
                sync.dma_start(out=out_d[:], in_=logits_sb[:]).then_inc(s_out, 16)
                sync.wait_ge(s_out, 16)

            @block.tensor
            def _(tensor):
                # Warmup fillers: ungate the PE clock (HAM) and keep it busy
                # until chunk 0 lands so the pooling burst runs at 2.4 GHz.
                tensor.wait_ge(s_warm, 1)
                for _ in range(14):
                    nc.tensor.matmul(
                        hps[0][:, :512], warm_sb[:, :128], warm_sb[:, :512],
                        start=True, stop=True,
                    )

                # Three concurrent matmul streams on disjoint PE column
                # strips (col_grps 0/1/2); they do NOT finish in program
                # order -- each stream's last matmul signals its own sem.
                tensor.wait_ge(s_member, 16)
                for k in range(n_slices):
                    ci = start_to_chunk.get(k)
                    if ci is not None:
                        tensor.wait_ge(s_chunk[ci], 16)
                        if ci == 1:
                            tensor.wait_ge(s_member1, 16)
                    lhsT = member_t[:, k * LOCAL_B : (k + 1) * LOCAL_B]
                    rs = k * H
                    for s in range(3):
                        mm = nc.tensor.matmul(
                            pooled[32 * s : 32 * (s + 1), :256],
                            lhsT,
                            hs_sb[:, rs + 256 * s : rs + 256 * (s + 1)],
                            start=(k == 0), stop=(k == n_slices - 1),
                        )
                        if k == n_slices - 1:
                            mm.then_inc(s_pool[s], 1)

                # transposes: pooledT[128h, 32b] per 128-col chunk, scaled by
                # diag(1/len) riding as the transpose's moving operand.
                tensor.wait_ge(s_smalls, 48)
                for c in range(6):
                    s = c // 2
                    tensor.wait_ge(s_sc[s], 2 if s == 1 else 1)
                    if c >= 3:
                        tensor.wait_ge(s_ptcopy, c - 2)
                    nc.tensor.transpose(
                        tp[c % 3][:, :LOCAL_B],
                        pooled_sb[32 * s : 32 * (s + 1), c * 128 : (c + 1) * 128],
                        identd_t[32 * s : 32 * (s + 1), :],
                    ).then_inc(s_tr, 1)

                # dense layer (fp16): hT[j, b] = tanh(db + dwT^T @ pooledT)
                tensor.wait_ge(s_ptcopy, 6)
                for jg in range(6):
                    tensor.wait_ge(s_dwTa if jg < 3 else s_dwTb, 16)
                    if jg >= 3:
                        tensor.wait_ge(s_tanh, jg - 2)
                    for c in range(6):
                        mm = nc.tensor.matmul(
                            hps[jg % 3][:, :LOCAL_B],
                            dwT_t[:, jg * H + c * 128 : jg * H + (c + 1) * 128],
                            pooledT_sb[:, c * LOCAL_B : (c + 1) * LOCAL_B],
                            start=(c == 0), stop=(c == 5),
                        )
                    mm.then_inc(s_head, 1)

                # classifier: logits[b, t] (hT chunk stationary -> batch-major)
                tensor.wait_ge(s_cwT, 16)
                for jg in range(6):
                    tensor.wait_ge(s_tanh, jg + 1)
                    mm = nc.tensor.matmul(
                        lps[:, :T_OUT],
                        hT_sb[:, jg * LOCAL_B : (jg + 1) * LOCAL_B],
                        cwT_t[:, jg * T_OUT : (jg + 1) * T_OUT],
                        start=(jg == 0), stop=(jg == 5),
                    )
                mm.then_inc(s_cls, 1)

            @block.vector
            def _(vector):
                # per-strip mean scale: pooled_sb[32s:32s+32, 256s:256s+256]
                # = psum strip s * (1/len) -- in/out stay on partitions
                # [32s, 32s+32), no cross-lane moves.
                vector.wait_ge(s_smalls, 48)
                vector.wait_ge(s_pool[0], 1)
                nc.vector.tensor_scalar_mul(
                    pooled_sb[0:32, 0:256],
                    pooled[0:32, :256],
                    smalls_t[0:32, 6:7],
                ).then_inc(s_sc[0], 1)
                # strip 1 is split: DVE scales its first half, ScalarE (after
                # its strip-2 copy) the second -- both run concurrently.
                vector.wait_ge(s_pool[1], 1)
                nc.vector.tensor_scalar_mul(
                    pooled_sb[32:64, 256:384],
                    pooled[32:64, :128],
                    smalls_t[32:64, 6:7],
                ).then_inc(s_sc[1], 1)
                for c in range(6):
                    vector.wait_ge(s_tr, c + 1)
                    nc.vector.tensor_copy(
                        pooledT_sb[:, c * LOCAL_B : (c + 1) * LOCAL_B],
                        tp[c % 3][:, :LOCAL_B],
                    ).then_inc(s_ptcopy, 1)
                vector.wait_ge(s_cls, 1)
                nc.vector.tensor_add(
                    logits_sb[:], lps[:, :T_OUT], clsb_t[:]
                ).then_inc(s_log, 1)

            @block.scalar
            def _(scalar):
                # Dummy tanh: pulls the lazy ACT_TABLE_LOAD (~1.3us) off the
                # critical path to kernel start.
                nc.scalar.activation(
                    scratch_sb[:, 0:1], warm_sb[:, 0:1],
                    mybir.ActivationFunctionType.Tanh,
                )
                # strip-2 scale-copy PSUM->SBUF on ScalarE (fast to PSUM;
                # concurrent with the DVE strip-0/1 scales; partitions 64-95).
                scalar.wait_ge(s_smalls, 48)
                scalar.wait_ge(s_pool[2], 1)
                nc.scalar.activation(
                    pooled_sb[64:96, 512:H], pooled[64:96, :256],
                    mybir.ActivationFunctionType.Copy,
                    scale=smalls_t[64:96, 6:7],
                ).then_inc(s_sc[2], 1)
                scalar.wait_ge(s_pool[1], 1)
                nc.scalar.activation(
                    pooled_sb[32:64, 384:512], pooled[32:64, 128:256],
                    mybir.ActivationFunctionType.Copy,
                    scale=smalls_t[32:64, 6:7],
                ).then_inc(s_sc[1], 1)
                for jg in range(6):
                    scalar.wait_ge(s_head, jg + 1)
                    nc.scalar.activation(
                        hT_sb[:, jg * LOCAL_B : (jg + 1) * LOCAL_B],
                        hps[jg % 3][:, :LOCAL_B],
                        mybir.ActivationFunctionType.Tanh,
                        bias=db6_ap[:, jg : jg + 1],
                        scale=(1.0 / 64.0) if dwt8 else 1.0,
                    ).then_inc(s_tanh, 1)

    nc.compile()
    return nc


def _ef_quantize(packed: np.ndarray, n_slices: int) -> np.ndarray:
    """Error-feedback quantization to fp8 e3m4, carried along the partition
    axis within each 128-row slice (= packed row order, sample-major), so each
    sample's device-side sum error collapses to its few chain-boundary
    carries."""
    arr = packed.reshape(n_slices, 128, H)
    q8 = np.empty((n_slices, 128, H), NP_F8)
    c = np.zeros((n_slices, H), np.float32)
    for p in range(128):
        y = arr[:, p, :] + c
        q = y.astype(NP_F8)
        c = y - q.astype(np.float32)
        q8[:, p, :] = q
    return q8


def kernel(hidden_states, pivot_len_list, dense_w, dense_b, cls_w, cls_b):
    global last_results
    hs = np.ascontiguousarray(np.asarray(hidden_states, dtype=np.float32))
    lens = np.asarray(pivot_len_list).astype(np.int64)
    dense_w = np.asarray(dense_w, dtype=np.float32)
    dense_b = np.asarray(dense_b, dtype=np.float32)
    cls_w = np.asarray(cls_w, dtype=np.float32)
    cls_b = np.asarray(cls_b, dtype=np.float32)
    assert hs.shape == (B, S, H), hs.shape
    assert lens.shape == (B,), lens.shape

    # ---- assign samples to cores: greedy LPT with a hard 32-per-core cap
    order = np.argsort(-lens, kind="stable")
    core_samples = [[] for _ in range(N_CORES)]
    load = np.zeros(N_CORES, dtype=np.int64)
    for b in order:
        open_cores = [c for c in range(N_CORES) if len(core_samples[c]) < LOCAL_B]
        c = min(open_cores, key=lambda c: load[c])
        core_samples[c].append(int(b))
        load[c] += int(lens[b])
    n_slices = max(2, -(-int(load.max()) // 128))

    key = (n_slices, COLT, DWT8)
    if key not in _cache:
        _cache[key] = _build_program(n_slices, COLT, DWT8)
    nc = _cache[key]

    # ---- shared (replicated) head tensors
    dwT_host = np.empty((128, 6 * H), np.float32)
    for jg in range(6):
        for c in range(6):
            dwT_host[:, jg * H + c * 128 : jg * H + (c + 1) * 128] = dense_w[
                jg * 128 : (jg + 1) * 128, c * 128 : (c + 1) * 128
            ].T
    cwT_host = np.empty((128, 6 * T_OUT), np.float32)
    for jg in range(6):
        cwT_host[:, jg * T_OUT : (jg + 1) * T_OUT] = cls_w[
            :, jg * 128 : (jg + 1) * 128
        ].T
    smalls_base = np.zeros((128, 8), np.float32)
    smalls_base[:, 0:6] = dense_b.reshape(6, 128).T
    clsb_host = np.ascontiguousarray(
        np.broadcast_to(cls_b, (LOCAL_B, T_OUT)).astype(np.float32)
    )
    identd_host = np.zeros((96, LOCAL_B), np.float16)
    for s in range(3):
        identd_host[32 * s : 32 * (s + 1)] = np.eye(LOCAL_B, dtype=np.float16)

    # ---- per-core packing
    hs2 = hs.reshape(B * S, H)
    NR = n_slices * 128
    in_maps = []
    for c in range(N_CORES):
        samples = core_samples[c]
        lens_c = lens[samples]
        idx = np.concatenate(
            [np.arange(b * S + 1, b * S + 1 + lens[b]) for b in samples]
        )
        n = idx.size
        packed = np.zeros((NR, H), np.float32)
        packed[:n] = hs2[idx]
        q8 = _ef_quantize(packed, n_slices)
        hsb_host = np.ascontiguousarray(
            q8.transpose(1, 0, 2).reshape(128, n_slices * H)
        )

        j = np.arange(n)
        kq = j // 128
        p = j % 128
        local_b = np.repeat(np.arange(LOCAL_B), lens_c)
        mem = np.zeros((128, n_slices * LOCAL_B), NP_F8)
        mem[p, kq * LOCAL_B + local_b] = NP_F8(1.0)

        invl = 1.0 / lens_c.astype(np.float32)
        smalls_host = smalls_base.copy()
        for s in range(3):                 # per-strip scale (partitions 32s+)
            smalls_host[32 * s : 32 * (s + 1), 6] = invl

        in_maps.append(
            {
                "hsb": hsb_host,
                "member": mem,
                "dwT": (np.clip(dwT_host * 64.0, -15.5, 15.5).astype(NP_F8)
                        if DWT8 else dwT_host.astype(np.float16)),
                "cwT": cwT_host.astype(np.float16),
                "identd": identd_host,
                "smalls": smalls_host,
                "clsb": clsb_host,
            }
        )

    trace = bool(os.environ.get("KERNEL_TRACE"))
    try:
        res = bass_utils.run_bass_kernel_spmd(
            nc, in_maps, list(range(N_CORES)), trace=trace
        )
    except Exception:
        # Transient NRT device errors clear on retry.
        res = bass_utils.run_bass_kernel_spmd(
            nc, in_maps, list(range(N_CORES)), trace=trace
        )
    last_results = res

    logits = np.zeros((B, T_OUT), np.float32)
    for c in range(N_CORES):
        logits[core_samples[c], :] = res.results[c]["out"].astype(np.float32)
    return logits
